# revision 2
# baseline (speedup 1.0000x reference)
"""Multi-head linear attention (elu+1 feature map) on 8 Trainium2 NeuronCores.

Problem: nn_MultiHeadLinearAttention — B=4, S=4096, H=16, D=64, E=1024.
    x = split_heads(query); q,k,v = per-head 64x64 projections of x
    phi = elu(.)+1;  kv = phi_k^T v (per head, summed over S); ksum = sum_s phi_k
    ctx = phi_q kv / (phi_q . ksum + eps);  out = combine_heads(ctx) @ Wo

Sharding (v2): core c handles batch b=c//2 and S-half h=c%2. Phase 1 computes
PARTIAL kv/ksum state for ALL 16 heads over the core's own S-half; the two
cores of a batch AllReduce-sum their partials (two collectives, one per
4-pair group, so the first exchange hides under the second group's compute).
A tiny warm-up collective at t=0 absorbs the CC-core rendezvous latency.
Phase 2 computes phi_q/ctx/output for the own S-half only.

All matmuls run in bf16 (f32 PSUM accumulation): 1 cycle/row at any N, half
the DMA/SBUF footprint, and far lower PE power than fp32r (which triggered
~50%-utilization power throttling on the baseline).

z is computed TRANSPOSED via ksum-stationary matmuls into quadrant rows
(32p..32p+1) of one PSUM tile — this replaces the baseline's 128 tiny N=2
matmuls + 32 PE transposes. 1/z is broadcast to 128 partitions with a
one-hot selector matmul (R), applied to phi_q (psc) before the ctx matmul.

phi(x) = elu(x)+1 = max(x+1, min(exp(x),1)): one ACT Exp + one fused DVE op.
"""

import sys

if "/opt/trn_rl_repo" not in sys.path:
    sys.path.insert(0, "/opt/trn_rl_repo")

import contextlib

import numpy as np
import ml_dtypes

import concourse.bass as bass
import concourse.tile as tile
from concourse import bacc, dve_ops, mybir
from concourse.bass_utils import run_bass_kernel_spmd
from concourse.dve_spec import Spec, Src0, Src1, One, maxx, minn
from concourse.dve_ops import RECIP_APPROX_FAST_CONSTS, RECIPROCAL_APPROX_FAST

F32 = mybir.dt.float32
F32R = mybir.dt.float32r
BF16 = mybir.dt.bfloat16
EXP = mybir.ActivationFunctionType.Exp
NPBF = ml_dtypes.bfloat16

B, S, H, D = 4, 4096, 16, 64
E = H * D              # 1024
SH = S // 2            # 2048 rows per core
P = 128                # partitions
NP = H // 2            # 8 head pairs
CH = 512               # free-dim chunk
NCH = SH // CH         # 4 chunks over own half
N_CORES = 8
REP = [[0, 1], [2, 3], [4, 5], [6, 7]]

_PHI_SHA = {"v3": "8446fb870b7054b2", "v4": None}


def _register_phi():
    for o in dve_ops.OPS:
        if o.name == "PHI_ELU1_ANT":
            return o
    op = dve_ops.DveOp(
        "PHI_ELU1_ANT",
        Spec(
            body=maxx(Src0 + One, minn(Src1, One)),
            reference=lambda in0, in1, c0, c1, c2: np.maximum(
                in0.astype(np.float32) + 1.0,
                np.minimum(in1.astype(np.float32), 1.0),
            ),
        ),
        subdim=False,
        uops_sha=dict(_PHI_SHA),
    )
    dve_ops.OPS.append(op)
    dve_ops.CUSTOM_DVE_SPECS[op.name] = op.spec
    dve_ops._SUB_OPCODE_FOR_NAME[op.name] = (
        max(dve_ops._SUB_OPCODE_FOR_NAME.values()) + 1
    )
    return op


def _build():
    phi_op = _register_phi()
    nc = bacc.Bacc("TRN2", target_bir_lowering=False, debug=False,
                   num_devices=N_CORES)

    xq_d = nc.dram_tensor("xq", [E, SH], BF16, kind="ExternalInput")
    wkv_d = nc.dram_tensor("wkv", [P, NP, 2 * P], BF16, kind="ExternalInput")
    wq_d = nc.dram_tensor("wq", [P, NP, P], BF16, kind="ExternalInput")
    wo_d = nc.dram_tensor("wo", [E, E], BF16, kind="ExternalInput")
    sel_d = nc.dram_tensor("sel", [P, P], F32R, kind="ExternalInput")
    yt_d = nc.dram_tensor("yt", [8, NCH, P, CH], BF16, kind="ExternalOutput")

    with tile.TileContext(nc) as tc:
        with contextlib.ExitStack() as ctx:
            persist = ctx.enter_context(tc.tile_pool(name="persist", bufs=1))
            dram = ctx.enter_context(
                tc.tile_pool(name="dram", bufs=1, space="DRAM"))

            # ---- warm-up collective: absorbs CC rendezvous latency ----
            warm_sb = persist.tile([2, 2], BF16, name="warmsb")
            nc.gpsimd.memset(warm_sb[:], 0.0)
            warm_in = dram.tile([2, 2], BF16, name="warmin")
            warm_out = dram.tile([2, 2], BF16, name="warmout")
            nc.sync.dma_start(warm_in[:], warm_sb[:])
            nc.gpsimd.collective_compute(
                "AllReduce", mybir.AluOpType.add, replica_groups=REP,
                ins=[warm_in[:].opt()], outs=[warm_out[:].opt()])

            # ---- persistent tiles + input DMA (phase-1 group 0 first) ----
            wkv_sb = persist.tile([P, NP, 2 * P], BF16, name="wkv")
            nc.sync.dma_start(wkv_sb[:], wkv_d[:, :, :])
            xq_sb = persist.tile([P, NP, SH], BF16, name="xq")
            for p in range(4):
                for c in range(NCH):
                    nc.sync.dma_start(
                        xq_sb[:, p, c * CH:(c + 1) * CH],
                        xq_d[p * P:(p + 1) * P, c * CH:(c + 1) * CH])
            for p in range(4, NP):
                for c in range(NCH):
                    nc.sync.dma_start(
                        xq_sb[:, p, c * CH:(c + 1) * CH],
                        xq_d[p * P:(p + 1) * P, c * CH:(c + 1) * CH])
            wq_sb = persist.tile([P, NP, P], BF16, name="wq")
            nc.sync.dma_start(wq_sb[:], wq_d[:, :, :])
            sel_sb = persist.tile([P, P], F32R, name="sel")
            nc.sync.dma_start(sel_sb[:], sel_d[:, :])

            # per-group state (post-collective): cols 0:128 kv, 128:160 ksum
            # (ksum zero-padded to 32 cols so the z matmul writes a full
            #  32-row PE quadrant -> z8 needs no separate zero-init)
            SW = P + 32
            st_sb = [persist.tile([P, 4, SW], BF16, name=f"st{g}")
                     for g in range(2)]
            sg_sb = [persist.tile([P, 4, SW], BF16, name=f"sg{g}")
                     for g in range(2)]
            # va: [v | 1 1] moving operand; ones cols written once
            va_sb = [persist.tile([P, 2, P + 2], BF16, name=f"va{i}")
                     for i in range(3)]
            for i in range(3):
                nc.gpsimd.memset(va_sb[i][:, :, P:P + 2], 1.0)
            phiq = persist.tile([P, NP, NCH, CH], BF16, name="phiq")

            stin = [dram.tile([P, 4, SW], BF16, name=f"stin{g}")
                    for g in range(2)]
            stout = [dram.tile([P, 4, SW], BF16, name=f"stout{g}")
                     for g in range(2)]

            # ================= PHASE 1: kv / ksum partials ==============
            with contextlib.ExitStack() as p1:
                kvps = p1.enter_context(
                    tc.tile_pool(name="kvps", bufs=4, space="PSUM"))
                accps = p1.enter_context(
                    tc.tile_pool(name="accps", bufs=1, space="PSUM"))
                ekp = p1.enter_context(tc.tile_pool(name="ekp", bufs=4))
                php = p1.enter_context(tc.tile_pool(name="php", bufs=4))

                for g in range(2):
                    acc = [accps.tile([P, P + 2], F32, name=f"acc{j}")
                           for j in range(4)]
                    units = [(c, p4, h) for c in range(NCH)
                             for p4 in range(4) for h in range(2)]
                    NU = len(units)

                    def proj(u):
                        c, p4, h = units[u]
                        p = 4 * g + p4
                        kv2 = kvps.tile([P, 2, 2 * P], F32, name="kv2")
                        for j in range(2):
                            si = 2 * h + j
                            nc.tensor.matmul(
                                kv2[:, j, :],
                                xq_sb[:, p, c * CH + si * P:c * CH + (si + 1) * P],
                                wkv_sb[:, p, :],
                                start=True, stop=True)
                        return kv2

                    def phik(u, kv2):
                        ek = ekp.tile([P, 2, P], BF16, name="ek")
                        nc.scalar.activation(ek[:], kv2[:, :, 0:P], EXP)
                        pht = php.tile([P, 2, P], BF16, name="ph")
                        nc.vector._custom_dve(
                            phi_op, out=pht[:], in0=kv2[:, :, 0:P], in1=ek[:])
                        va = va_sb[u % 3]
                        if u % 2 == 0:
                            nc.scalar.copy(va[:, :, 0:P], kv2[:, :, P:2 * P])
                        else:
                            nc.vector.tensor_copy(va[:, :, 0:P],
                                                  kv2[:, :, P:2 * P])
                        return pht, va

                    def accm(u, pht, va):
                        c, p4, h = units[u]
                        for j in range(2):
                            nc.tensor.matmul(
                                acc[p4][:],
                                pht[:, j, :],
                                va[:, j, :],
                                start=(c == 0 and h == 0 and j == 0),
                                stop=(c == NCH - 1 and h == 1 and j == 1))

                    # software pipeline, lookahead 3
                    LA = 4
                    kvs = {u: proj(u) for u in range(min(LA, NU))}
                    for u in range(NU):
                        ph = phik(u, kvs.pop(u))
                        if u + LA < NU:
                            kvs[u + LA] = proj(u + LA)
                        accm(u, *ph)

                    # evict group state: block-diag kv + zero-padded ksum
                    sg = sg_sb[g]
                    nc.gpsimd.memset(sg[:], 0.0)
                    for p4 in range(4):
                        a = acc[p4]
                        if p4 % 2 == 0:
                            nc.scalar.copy(sg[0:D, p4, 0:D], a[0:D, 0:D])
                            nc.scalar.copy(sg[0:D, p4, P:P + 1],
                                           a[0:D, P:P + 1])
                            nc.vector.tensor_copy(sg[D:P, p4, D:P],
                                                  a[D:P, D:P])
                            nc.vector.tensor_copy(sg[D:P, p4, P + 1:P + 2],
                                                  a[D:P, P + 1:P + 2])
                        else:
                            nc.vector.tensor_copy(sg[0:D, p4, 0:D],
                                                  a[0:D, 0:D])
                            nc.vector.tensor_copy(sg[0:D, p4, P:P + 1],
                                                  a[0:D, P:P + 1])
                            nc.scalar.copy(sg[D:P, p4, D:P], a[D:P, D:P])
                            nc.scalar.copy(sg[D:P, p4, P + 1:P + 2],
                                           a[D:P, P + 1:P + 2])
                    nc.sync.dma_start(stin[g][:], sg[:])
                    nc.gpsimd.collective_compute(
                        "AllReduce", mybir.AluOpType.add, replica_groups=REP,
                        ins=[stin[g][:].opt()], outs=[stout[g][:].opt()])

                    if g == 0:
                        # load Wo while group 1 computes
                        wo_sb = persist.tile([P, 8, E], BF16, name="wo")
                        for e in range(8):
                            nc.sync.dma_start(wo_sb[:, e, :],
                                              wo_d[e * P:(e + 1) * P, :])

            # state load-back (gated on the collectives)
            for g in range(2):
                nc.sync.dma_start(st_sb[g][:], stout[g][:])

            # ================= PHASE 2: q / ctx / output ================
            # per chunk: q-proj (4 pairs) -> z -> (lag-1) R/psc/ctx; group a
            # chunks first, then group b (group-b state lands ~15us later).
            # -- 2a: phi_q for all pairs/chunks (no collective dependency)
            with contextlib.ExitStack() as p2a:
                qtps = p2a.enter_context(
                    tc.tile_pool(name="qtps", bufs=6, space="PSUM"))
                eqp = p2a.enter_context(tc.tile_pool(name="eqp", bufs=3))
                for c in range(NCH):
                    for p in range(NP):
                        qt = qtps.tile([P, CH], F32, name="qt")
                        nc.tensor.matmul(qt[:], wq_sb[:, p, :],
                                         xq_sb[:, p, c * CH:(c + 1) * CH],
                                         start=True, stop=True)
                        eq = eqp.tile([P, CH], BF16, name="eq")
                        nc.scalar.activation(eq[:], qt[:], EXP)
                        nc.vector._custom_dve(
                            phi_op, out=phiq[:, p, c, :], in0=qt[:], in1=eq[:])

            with contextlib.ExitStack() as p2:
                zps = p2.enter_context(
                    tc.tile_pool(name="zps", bufs=2, space="PSUM"))
                rps = p2.enter_context(
                    tc.tile_pool(name="rps", bufs=4, space="PSUM"))
                ctps = p2.enter_context(
                    tc.tile_pool(name="ctps", bufs=2, space="PSUM"))
                zrp = p2.enter_context(tc.tile_pool(name="zrp", bufs=8))
                pscp = p2.enter_context(tc.tile_pool(name="pscp", bufs=5))
                ctsp = p2.enter_context(tc.tile_pool(name="ctsp", bufs=4))

                rc = RECIP_APPROX_FAST_CONSTS

                def zgrp(g, c):
                    z8 = zps.tile([P, CH], F32, name="z8")
                    for p4 in range(4):
                        # 32-wide ksum stationary writes the full quadrant
                        nc.tensor.matmul(
                            z8[32 * p4:32 * p4 + 32, :],
                            st_sb[g][:, p4, P:P + 32],
                            phiq[:, 4 * g + p4, c, :],
                            start=True, stop=True,
                            tile_position=(0, 32 * p4))
                    zr = zrp.tile([P, CH], F32R, name="zr")
                    nc.vector._custom_dve(
                        RECIPROCAL_APPROX_FAST, out=zr[:], in0=z8[:],
                        s0=rc["s0"], s1=rc["s1"], imm2=rc["imm2"])
                    return zr

                def ctx_chunk(g, c, zr, cts_t):
                    # pass 1: all four R broadcasts + psc (DVE) ...
                    pscs = []
                    for p4 in range(4):
                        p = 4 * g + p4
                        R = rps.tile([P, CH], F32, name="R")
                        nc.tensor.matmul(R[:], sel_sb[32 * p4:32 * p4 + 2, :],
                                         zr[32 * p4:32 * p4 + 2, :],
                                         start=True, stop=True,
                                         tile_position=(32 * p4, 0))
                        psc = pscp.tile([P, CH], BF16, name="psc")
                        nc.vector.tensor_mul(psc[:], phiq[:, p, c, :], R[:])
                        pscs.append(psc)
                    # ... pass 2: ctx matmuls (psc computed under later Rs)
                    for p4 in range(4):
                        p = 4 * g + p4
                        ct = ctps.tile([P, CH], F32, name="ct")
                        nc.tensor.matmul(ct[:], st_sb[g][:, p4, 0:P],
                                         pscs[p4][:], start=True, stop=True)
                        nc.scalar.copy(cts_t[:, p, :], ct[:])

                cts_t = [ctsp.tile([P, NP, CH], BF16, name="cts")
                         for _ in range(NCH)]
                # all z first (recip latency hides under later z matmuls)
                zra = [zgrp(0, c) for c in range(NCH)]
                zrb = [zgrp(1, c) for c in range(NCH)]
                for c in range(NCH):
                    ctx_chunk(0, c, zra[c], cts_t[c])
                for c in range(NCH):
                    ctx_chunk(1, c, zrb[c], cts_t[c])

            # Wo projection: one dense tensor-only block (ACT/DVE quiet
            # -> the power governor lifts the 50% PE-utilization cap)
            with contextlib.ExitStack() as p3:
                yps = p3.enter_context(
                    tc.tile_pool(name="yps", bufs=4, space="PSUM"))
                ysp = p3.enter_context(tc.tile_pool(name="ysp", bufs=4))
                for c in range(NCH):
                    for o in range(8):
                        yo = yps.tile([P, CH], F32, name="yo")
                        for e in range(8):
                            nc.tensor.matmul(
                                yo[:],
                                wo_sb[:, e, o * P:(o + 1) * P],
                                cts_t[c][:, e, :],
                                start=(e == 0), stop=(e == 7))
                        ys = ysp.tile([P, CH], BF16, name="ys")
                        with nc.allow_low_precision("bf16 out, 2e-2 budget"):
                            if o % 2 == 0:
                                nc.scalar.copy(ys[:], yo[:])
                            else:
                                nc.vector.tensor_copy(ys[:], yo[:])
                        nc.sync.dma_start(yt_d[o, c, :, :], ys[:])

    nc.compile()
    return nc


_CACHED_NC = None


def _get_nc():
    global _CACHED_NC
    if _CACHED_NC is None:
        _CACHED_NC = _build()
    return _CACHED_NC


def _host_inputs(query, Wq, Wk, Wv, Wo):
    """Build the 8 per-core input maps (host-side prep, not timed)."""
    query = np.asarray(query, dtype=np.float32)
    Wq = np.asarray(Wq, dtype=np.float32)
    Wk = np.asarray(Wk, dtype=np.float32)
    Wv = np.asarray(Wv, dtype=np.float32)
    Wo = np.asarray(Wo, dtype=np.float32)

    wkv = np.zeros((P, NP, 2 * P), dtype=NPBF)
    wq = np.zeros((P, NP, P), dtype=NPBF)
    for p in range(NP):
        for j in range(2):
            h = 2 * p + j
            sl = slice(j * D, (j + 1) * D)
            wkv[sl, p, j * D:(j + 1) * D] = Wk[h]
            wkv[sl, p, P + j * D:P + (j + 1) * D] = Wv[h]
            wq[sl, p, j * D:(j + 1) * D] = Wq[h]
    sel = np.zeros((P, P), dtype=np.float32)
    for q in range(4):
        sel[32 * q, 0:D] = 1.0
        sel[32 * q + 1, D:P] = 1.0
    wo_b = Wo.astype(NPBF)

    in_maps = []
    for c in range(N_CORES):
        b, half = c // 2, c % 2
        xq = np.ascontiguousarray(
            query[b, half * SH:(half + 1) * SH, :].T.astype(NPBF))
        in_maps.append({
            "xq": xq, "wkv": wkv, "wq": wq, "wo": wo_b, "sel": sel,
        })
    return in_maps


def _run(in_maps, trace=False):
    nc = _get_nc()
    return run_bass_kernel_spmd(nc, in_maps, core_ids=list(range(N_CORES)),
                                trace=trace)


def _assemble(res):
    out = np.empty((B, S, E), dtype=np.float32)
    for c in range(N_CORES):
        b, half = c // 2, c % 2
        yt = np.asarray(res.results[c]["yt"])  # [8, NCH, P, CH] bf16
        out[b, half * SH:(half + 1) * SH, :] = (
            yt.transpose(1, 3, 0, 2).reshape(SH, E).astype(np.float32))
    return out


def kernel(query, Wq, Wk, Wv, Wo):
    in_maps = _host_inputs(query, Wq, Wk, Wv, Wo)
    res = _run(in_maps)
    return _assemble(res)


# revision 3
# speedup vs baseline: 1.1881x; 1.1881x over previous
"""Multi-head linear attention (elu+1 feature map) on 8 Trainium2 NeuronCores.

Problem: nn_MultiHeadLinearAttention — B=4, S=4096, H=16, D=64, E=1024.
    x = split_heads(query); q,k,v = per-head 64x64 projections of x
    phi = elu(.)+1;  kv = phi_k^T v (per head, summed over S); ksum = sum_s phi_k
    ctx = phi_q kv / (phi_q . ksum + eps);  out = combine_heads(ctx) @ Wo

Sharding (v2): core c handles batch b=c//2 and S-half h=c%2. Phase 1 computes
PARTIAL kv/ksum state for ALL 16 heads over the core's own S-half; the two
cores of a batch AllReduce-sum their partials (two collectives, one per
4-pair group, so the first exchange hides under the second group's compute).
A tiny warm-up collective at t=0 absorbs the CC-core rendezvous latency.
Phase 2 computes phi_q/ctx/output for the own S-half only.

All matmuls run in bf16 (f32 PSUM accumulation): 1 cycle/row at any N, half
the DMA/SBUF footprint, and far lower PE power than fp32r (which triggered
~50%-utilization power throttling on the baseline).

z is computed TRANSPOSED via ksum-stationary matmuls into quadrant rows
(32p..32p+1) of one PSUM tile — this replaces the baseline's 128 tiny N=2
matmuls + 32 PE transposes. 1/z is broadcast to 128 partitions with a
one-hot selector matmul (R), applied to phi_q (psc) before the ctx matmul.

phi(x) = elu(x)+1 = max(x+1, min(exp(x),1)): one ACT Exp + one fused DVE op.
"""

import sys

if "/opt/trn_rl_repo" not in sys.path:
    sys.path.insert(0, "/opt/trn_rl_repo")

import contextlib

import numpy as np
import ml_dtypes

import concourse.bass as bass
import concourse.tile as tile
from concourse import bacc, dve_ops, mybir
from concourse.bass_utils import run_bass_kernel_spmd
from concourse.dve_spec import Spec, Src0, Src1, One, maxx, minn
from concourse.dve_ops import RECIP_APPROX_FAST_CONSTS, RECIPROCAL_APPROX_FAST

F32 = mybir.dt.float32
F32R = mybir.dt.float32r
BF16 = mybir.dt.bfloat16
EXP = mybir.ActivationFunctionType.Exp
NPBF = ml_dtypes.bfloat16

B, S, H, D = 4, 4096, 16, 64
E = H * D              # 1024
SH = S // 2            # 2048 rows per core
P = 128                # partitions
NP = H // 2            # 8 head pairs
CH = 512               # free-dim chunk
NCH = SH // CH         # 4 chunks over own half
N_CORES = 8
REP = [[0, 1], [2, 3], [4, 5], [6, 7]]

_PHI_SHA = {"v3": "8446fb870b7054b2", "v4": None}


def _register_phi():
    for o in dve_ops.OPS:
        if o.name == "PHI_ELU1_ANT":
            return o
    op = dve_ops.DveOp(
        "PHI_ELU1_ANT",
        Spec(
            body=maxx(Src0 + One, minn(Src1, One)),
            reference=lambda in0, in1, c0, c1, c2: np.maximum(
                in0.astype(np.float32) + 1.0,
                np.minimum(in1.astype(np.float32), 1.0),
            ),
        ),
        subdim=False,
        uops_sha=dict(_PHI_SHA),
    )
    dve_ops.OPS.append(op)
    dve_ops.CUSTOM_DVE_SPECS[op.name] = op.spec
    dve_ops._SUB_OPCODE_FOR_NAME[op.name] = (
        max(dve_ops._SUB_OPCODE_FOR_NAME.values()) + 1
    )
    return op


def _build():
    phi_op = _register_phi()
    nc = bacc.Bacc("TRN2", target_bir_lowering=False, debug=False,
                   num_devices=N_CORES)

    xq_d = nc.dram_tensor("xq", [E, SH], BF16, kind="ExternalInput")
    wkv_d = nc.dram_tensor("wkv", [P, NP, 2 * P], BF16, kind="ExternalInput")
    wq_d = nc.dram_tensor("wq", [P, NP, P], BF16, kind="ExternalInput")
    wo_d = nc.dram_tensor("wo", [E, E], BF16, kind="ExternalInput")
    sel_d = nc.dram_tensor("sel", [P, P], F32R, kind="ExternalInput")
    yt_d = nc.dram_tensor("yt", [8, NCH, P, CH], BF16, kind="ExternalOutput")

    with tile.TileContext(nc) as tc:
        with contextlib.ExitStack() as ctx:
            persist = ctx.enter_context(tc.tile_pool(name="persist", bufs=1))
            dram = ctx.enter_context(
                tc.tile_pool(name="dram", bufs=1, space="DRAM"))

            # ---- warm-up collective: absorbs CC rendezvous latency ----
            warm_sb = persist.tile([2, 2], BF16, name="warmsb")
            nc.gpsimd.memset(warm_sb[:], 0.0)
            warm_in = dram.tile([2, 2], BF16, name="warmin")
            warm_out = dram.tile([2, 2], BF16, name="warmout")
            nc.sync.dma_start(warm_in[:], warm_sb[:])
            nc.gpsimd.collective_compute(
                "AllReduce", mybir.AluOpType.add, replica_groups=REP,
                ins=[warm_in[:].opt()], outs=[warm_out[:].opt()])

            # ---- persistent tiles + input DMA (phase-1 group 0 first) ----
            wkv_sb = persist.tile([P, NP, 2 * P], BF16, name="wkv")
            nc.sync.dma_start(wkv_sb[:], wkv_d[:, :, :])
            xq_sb = persist.tile([P, NP, SH], BF16, name="xq")
            for p in range(4):
                for c in range(NCH):
                    nc.sync.dma_start(
                        xq_sb[:, p, c * CH:(c + 1) * CH],
                        xq_d[p * P:(p + 1) * P, c * CH:(c + 1) * CH])
            for p in range(4, NP):
                for c in range(NCH):
                    nc.sync.dma_start(
                        xq_sb[:, p, c * CH:(c + 1) * CH],
                        xq_d[p * P:(p + 1) * P, c * CH:(c + 1) * CH])
            wq_sb = persist.tile([P, NP, P], BF16, name="wq")
            nc.sync.dma_start(wq_sb[:], wq_d[:, :, :])
            sel_sb = persist.tile([P, P], F32R, name="sel")
            nc.sync.dma_start(sel_sb[:], sel_d[:, :])

            # per-group state (post-collective): cols 0:128 kv, 128:160 ksum
            # (ksum zero-padded to 32 cols so the z matmul writes a full
            #  32-row PE quadrant -> z8 needs no separate zero-init)
            SW = P + 32
            st_sb = [persist.tile([P, 4, SW], BF16, name=f"st{g}")
                     for g in range(2)]
            sg_sb = [persist.tile([P, 4, SW], BF16, name=f"sg{g}")
                     for g in range(2)]
            # va: [v | 1 1] moving operand; ones cols written once
            va_sb = [persist.tile([P, 2, P + 2], BF16, name=f"va{i}")
                     for i in range(4)]
            for i in range(4):
                nc.gpsimd.memset(va_sb[i][:, :, P:P + 2], 1.0)
            phiq = persist.tile([P, NP, NCH, CH], BF16, name="phiq")

            stin = [dram.tile([P, 4, SW], BF16, name=f"stin{g}")
                    for g in range(2)]
            stout = [dram.tile([P, 4, SW], BF16, name=f"stout{g}")
                     for g in range(2)]

            # ================= PHASE 1: kv / ksum partials ==============
            with contextlib.ExitStack() as p1:
                kvps = p1.enter_context(
                    tc.tile_pool(name="kvps", bufs=4, space="PSUM"))
                accps = p1.enter_context(
                    tc.tile_pool(name="accps", bufs=1, space="PSUM"))
                ekp = p1.enter_context(tc.tile_pool(name="ekp", bufs=6))
                php = p1.enter_context(tc.tile_pool(name="php", bufs=6))

                for g in range(2):
                    acc = [accps.tile([P, P + 2], F32, name=f"acc{j}")
                           for j in range(4)]
                    units = [(c, p4, h) for c in range(NCH)
                             for p4 in range(4) for h in range(2)]
                    NU = len(units)

                    def proj(u):
                        c, p4, h = units[u]
                        p = 4 * g + p4
                        kv2 = kvps.tile([P, 2, 2 * P], F32, name="kv2")
                        for j in range(2):
                            si = 2 * h + j
                            nc.tensor.matmul(
                                kv2[:, j, :],
                                xq_sb[:, p, c * CH + si * P:c * CH + (si + 1) * P],
                                wkv_sb[:, p, :],
                                start=True, stop=True)
                        return kv2

                    def phik(u, kv2):
                        ek = ekp.tile([P, 2, P], BF16, name="ek")
                        nc.scalar.activation(ek[:], kv2[:, :, 0:P], EXP)
                        pht = php.tile([P, 2, P], BF16, name="ph")
                        nc.vector._custom_dve(
                            phi_op, out=pht[:], in0=kv2[:, :, 0:P], in1=ek[:])
                        va = va_sb[u % 4]
                        if u % 2 == 0:
                            nc.scalar.copy(va[:, :, 0:P], kv2[:, :, P:2 * P])
                        else:
                            nc.vector.tensor_copy(va[:, :, 0:P],
                                                  kv2[:, :, P:2 * P])
                        return pht, va

                    def accm(u, pht, va):
                        c, p4, h = units[u]
                        for j in range(2):
                            nc.tensor.matmul(
                                acc[p4][:],
                                pht[:, j, :],
                                va[:, j, :],
                                start=(c == 0 and h == 0 and j == 0),
                                stop=(c == NCH - 1 and h == 1 and j == 1))

                    # software pipeline, lookahead 3
                    LA = 4
                    kvs = {u: proj(u) for u in range(min(LA, NU))}
                    for u in range(NU):
                        ph = phik(u, kvs.pop(u))
                        if u + LA < NU:
                            kvs[u + LA] = proj(u + LA)
                        accm(u, *ph)

                    # evict group state: block-diag kv + zero-padded ksum
                    sg = sg_sb[g]
                    nc.gpsimd.memset(sg[:], 0.0)
                    for p4 in range(4):
                        a = acc[p4]
                        if p4 % 2 == 0:
                            nc.scalar.copy(sg[0:D, p4, 0:D], a[0:D, 0:D])
                            nc.scalar.copy(sg[0:D, p4, P:P + 1],
                                           a[0:D, P:P + 1])
                            nc.vector.tensor_copy(sg[D:P, p4, D:P],
                                                  a[D:P, D:P])
                            nc.vector.tensor_copy(sg[D:P, p4, P + 1:P + 2],
                                                  a[D:P, P + 1:P + 2])
                        else:
                            nc.vector.tensor_copy(sg[0:D, p4, 0:D],
                                                  a[0:D, 0:D])
                            nc.vector.tensor_copy(sg[0:D, p4, P:P + 1],
                                                  a[0:D, P:P + 1])
                            nc.scalar.copy(sg[D:P, p4, D:P], a[D:P, D:P])
                            nc.scalar.copy(sg[D:P, p4, P + 1:P + 2],
                                           a[D:P, P + 1:P + 2])
                    nc.sync.dma_start(stin[g][:], sg[:])
                    nc.gpsimd.collective_compute(
                        "AllReduce", mybir.AluOpType.add, replica_groups=REP,
                        ins=[stin[g][:].opt()], outs=[stout[g][:].opt()])

                    if g == 0:
                        # load Wo while group 1 computes
                        wo_sb = persist.tile([P, 8, E], BF16, name="wo")
                        for e in range(8):
                            nc.sync.dma_start(wo_sb[:, e, :],
                                              wo_d[e * P:(e + 1) * P, :])

            # state load-back (gated on the collectives)
            for g in range(2):
                nc.sync.dma_start(st_sb[g][:], stout[g][:])

            # ================= PHASE 2: q / ctx / output ================
            # per chunk: q-proj (4 pairs) -> z -> (lag-1) R/psc/ctx; group a
            # chunks first, then group b (group-b state lands ~15us later).
            # -- 2a: phi_q for all pairs/chunks (no collective dependency)
            with contextlib.ExitStack() as p2a:
                qtps = p2a.enter_context(
                    tc.tile_pool(name="qtps", bufs=6, space="PSUM"))
                eqp = p2a.enter_context(tc.tile_pool(name="eqp", bufs=3))
                for c in range(NCH):
                    for p in range(NP):
                        qt = qtps.tile([P, CH], F32, name="qt")
                        nc.tensor.matmul(qt[:], wq_sb[:, p, :],
                                         xq_sb[:, p, c * CH:(c + 1) * CH],
                                         start=True, stop=True)
                        eq = eqp.tile([P, CH], BF16, name="eq")
                        nc.scalar.activation(eq[:], qt[:], EXP)
                        nc.vector._custom_dve(
                            phi_op, out=phiq[:, p, c, :], in0=qt[:], in1=eq[:])

            with contextlib.ExitStack() as p2:
                zps = p2.enter_context(
                    tc.tile_pool(name="zps", bufs=2, space="PSUM"))
                rps = p2.enter_context(
                    tc.tile_pool(name="rps", bufs=4, space="PSUM"))
                ctps = p2.enter_context(
                    tc.tile_pool(name="ctps", bufs=2, space="PSUM"))
                zrp = p2.enter_context(tc.tile_pool(name="zrp", bufs=8))
                pscp = p2.enter_context(tc.tile_pool(name="pscp", bufs=5))
                ctsp = p2.enter_context(tc.tile_pool(name="ctsp", bufs=4))

                rc = RECIP_APPROX_FAST_CONSTS

                def zgrp(g, c):
                    z8 = zps.tile([P, CH], F32, name="z8")
                    for p4 in range(4):
                        # 32-wide ksum stationary writes the full quadrant
                        nc.tensor.matmul(
                            z8[32 * p4:32 * p4 + 32, :],
                            st_sb[g][:, p4, P:P + 32],
                            phiq[:, 4 * g + p4, c, :],
                            start=True, stop=True,
                            tile_position=(0, 32 * p4))
                    zr = zrp.tile([P, CH], F32R, name="zr")
                    nc.vector._custom_dve(
                        RECIPROCAL_APPROX_FAST, out=zr[:], in0=z8[:],
                        s0=rc["s0"], s1=rc["s1"], imm2=rc["imm2"])
                    return zr

                def ctx_chunk(g, c, zr, cts_t):
                    # pass 1: all four R broadcasts + psc (DVE) ...
                    pscs = []
                    for p4 in range(4):
                        p = 4 * g + p4
                        R = rps.tile([P, CH], F32, name="R")
                        nc.tensor.matmul(R[:], sel_sb[32 * p4:32 * p4 + 2, :],
                                         zr[32 * p4:32 * p4 + 2, :],
                                         start=True, stop=True,
                                         tile_position=(32 * p4, 0))
                        psc = pscp.tile([P, CH], BF16, name="psc")
                        nc.vector.tensor_mul(psc[:], phiq[:, p, c, :], R[:])
                        pscs.append(psc)
                    # ... pass 2: ctx matmuls (psc computed under later Rs)
                    for p4 in range(4):
                        p = 4 * g + p4
                        ct = ctps.tile([P, CH], F32, name="ct")
                        nc.tensor.matmul(ct[:], st_sb[g][:, p4, 0:P],
                                         pscs[p4][:], start=True, stop=True)
                        nc.scalar.copy(cts_t[:, p, :], ct[:])

                cts_t = [ctsp.tile([P, NP, CH], BF16, name="cts")
                         for _ in range(NCH)]
                # all z first (recip latency hides under later z matmuls)
                zra = [zgrp(0, c) for c in range(NCH)]
                zrb = [zgrp(1, c) for c in range(NCH)]
                for c in range(NCH):
                    ctx_chunk(0, c, zra[c], cts_t[c])
                for c in range(NCH):
                    ctx_chunk(1, c, zrb[c], cts_t[c])

            # Wo projection: one dense tensor-only block (ACT/DVE quiet
            # -> the power governor lifts the 50% PE-utilization cap)
            with contextlib.ExitStack() as p3:
                yps = p3.enter_context(
                    tc.tile_pool(name="yps", bufs=4, space="PSUM"))
                ysp = p3.enter_context(tc.tile_pool(name="ysp", bufs=4))
                for c in range(NCH):
                    for o in range(8):
                        yo = yps.tile([P, CH], F32, name="yo")
                        for e in range(8):
                            nc.tensor.matmul(
                                yo[:],
                                wo_sb[:, e, o * P:(o + 1) * P],
                                cts_t[c][:, e, :],
                                start=(e == 0), stop=(e == 7))
                        ys = ysp.tile([P, CH], BF16, name="ys")
                        with nc.allow_low_precision("bf16 out, 2e-2 budget"):
                            if o % 2 == 0:
                                nc.scalar.copy(ys[:], yo[:])
                            else:
                                nc.vector.tensor_copy(ys[:], yo[:])
                        nc.sync.dma_start(yt_d[o, c, :, :], ys[:])

    nc.compile()
    return nc


_CACHED_NC = None


def _get_nc():
    global _CACHED_NC
    if _CACHED_NC is None:
        _CACHED_NC = _build()
    return _CACHED_NC


def _host_inputs(query, Wq, Wk, Wv, Wo):
    """Build the 8 per-core input maps (host-side prep, not timed)."""
    query = np.asarray(query, dtype=np.float32)
    Wq = np.asarray(Wq, dtype=np.float32)
    Wk = np.asarray(Wk, dtype=np.float32)
    Wv = np.asarray(Wv, dtype=np.float32)
    Wo = np.asarray(Wo, dtype=np.float32)

    wkv = np.zeros((P, NP, 2 * P), dtype=NPBF)
    wq = np.zeros((P, NP, P), dtype=NPBF)
    for p in range(NP):
        for j in range(2):
            h = 2 * p + j
            sl = slice(j * D, (j + 1) * D)
            wkv[sl, p, j * D:(j + 1) * D] = Wk[h]
            wkv[sl, p, P + j * D:P + (j + 1) * D] = Wv[h]
            wq[sl, p, j * D:(j + 1) * D] = Wq[h]
    sel = np.zeros((P, P), dtype=np.float32)
    for q in range(4):
        sel[32 * q, 0:D] = 1.0
        sel[32 * q + 1, D:P] = 1.0
    wo_b = Wo.astype(NPBF)

    in_maps = []
    for c in range(N_CORES):
        b, half = c // 2, c % 2
        xq = np.ascontiguousarray(
            query[b, half * SH:(half + 1) * SH, :].T.astype(NPBF))
        in_maps.append({
            "xq": xq, "wkv": wkv, "wq": wq, "wo": wo_b, "sel": sel,
        })
    return in_maps


def _run(in_maps, trace=False):
    nc = _get_nc()
    return run_bass_kernel_spmd(nc, in_maps, core_ids=list(range(N_CORES)),
                                trace=trace)


def _assemble(res):
    out = np.empty((B, S, E), dtype=np.float32)
    for c in range(N_CORES):
        b, half = c // 2, c % 2
        yt = np.asarray(res.results[c]["yt"])  # [8, NCH, P, CH] bf16
        out[b, half * SH:(half + 1) * SH, :] = (
            yt.transpose(1, 3, 0, 2).reshape(SH, E).astype(np.float32))
    return out


def kernel(query, Wq, Wk, Wv, Wo):
    in_maps = _host_inputs(query, Wq, Wk, Wv, Wo)
    res = _run(in_maps)
    return _assemble(res)
